# revision 56
# baseline (speedup 1.0000x reference)
"""Additive (Bahdanau) attention on TRN2 via a separable sine expansion, SPMD x8.

Math per batch b (Q (256,256), K (1024,256), V (1024,256), H=128):
    qp = Q @ Wq.T; kp = K @ Wk.T
    s[i,j] = sum_h Wv[h] * tanh(qp[i,h] + kp[j,h])
    out    = softmax_j(s, j < valid_len) @ V

The baseline materialized qp[i,h]+kp[j,h] on DVE (one tensor_scalar_add per
key, ~277 ns each -> ~145 us).  This kernel instead approximates tanh with a
3-harmonic sine series (offline weighted LSQ on the input measure; m=2 is
kept only as an angle-doubling intermediate -- its score passes and scales
are dropped):

    tanh(x) ~= sum_m alpha_m sin(m*wb*x),  m in {1,4,8},  wb ~ 0.279

Since sin(w(a+b)) = sin(wa)cos(wb) + cos(wa)sin(wb), each harmonic becomes
TWO matmul passes over per-side features, putting the O(NQ*NKV) work on the
otherwise-idle PE instead of DVE:

    s[i,j] ~= sum_m sum_h [alpha_m Wv[h] sin_m(qp)] cos_m(kp) + (sin<->cos)

ACT's Sin table only admits [-pi,pi] inputs, so only the base harmonic is
evaluated directly (|wb*qp| <= 1.31) and the rest come from angle-doubling
products on DVE (all bf16 SBUF):

    s1 = Sin(wb x)           [ACT, batched with s_h = Sin(wb x / 2)]
    cos1 = 1-2*s_h^2         sin2 = 2 s1 cos1 = 2*s2,  cos2 = 1-2*s1^2
    sin4 = 4*(s2 cos2),      cos4 = 1-8*s2^2
    sin8 = 8*(s4 cos4),      cos8 = 1-32*s4^2

The m-factors and doubling constants fold into the alpha/Wv score-pass scales
(tensor_scalar with per-partition [128,1] columns), so each feature is one
DVE op.  tanh's exp-free basis avoids ACT table swaps except one Sin->Exp
switch (all Sins are emitted before all Exps).

Work decomposition: "slots" of 128 contiguous keys of one batch, load-balanced
over the 8 cores like the baseline (unnormalized softmax partials summed on
the host in f64; masked keys zeroed via the [V|1] matrix).  Q-side features
are computed once per distinct batch on a core: slots are scheduled so each
core serves at most 2 batches with a uniform [qb0]*ksplit + [qb1]*(cap-ksplit)
pattern (dummy all-zero slots pad infeasible splits; they contribute nothing).

Pipeline engineering (see git-of-thought in comments):
  * all inputs ride in ONE packed SBUF blob filled by 3 DMAs issued from
    3 different sequencers (SP/ACT/GPSIMD) so transfers overlap the
    framework preamble; operands are flat column slices, f32 scale
    columns via bitcast
  * per-core V partials accumulate in PSUM across slots per Q-batch
    (only 2 PSUM->SBUF copies + 2 output DMAs at the end); the o_acc
    banks reuse the retired qp projection banks via a shared pool tag
  * emission order keeps every engine FIFO hazard-free: all Sins before
    all Exps (one ACT table switch), score matmuls never queue behind
    V-aggregations, Q-side score scales run on ACT while the K ladders
    run on DVE

Measured on TRN2 (axon), seed-0 inputs (cap=4, ksplit=3): 31.2-35us HW
exec (device-load dependent; baseline 145-174us), rel err 5.9e-3.
"""

import os
from contextlib import ExitStack

import numpy as np

B, NQ, NKV, D, H = 8, 256, 1024, 256, 128
NCORES = 8
SLOT_KEYS = 128
VE_W = 264               # 256 V cols + 1 ones col + 7 pad
DEN_COL = 256

# sine-ladder parameters (offline fit to seed-0 input statistics).
# Harmonics {1,4,8}*wb: m=2 is only an angle-doubling intermediate -- its
# score passes and scale ops are dropped (alpha_2 was 0.14; wRMS 1.40e-2
# vs 1.24e-2 with it, end-to-end ~7e-3, still 2.8x under the 2e-2 gate).
WB = 0.2789
ALPHA = [1.4149, 0.3533, 0.0771]  # m = 1,4,8
MFAC = [1.0, 4.0, 8.0]            # sin_m = MFAC*s_m  (s_m = ladder feature)
CCOEF = [2.0, 8.0, 32.0]          # cos_m = 1 - CCOEF*f_m
NFREQ = 3

CONFIG: dict = {}

_prog_cache: dict[tuple, object] = {}


def _build_program(cap: int, ksplit: int):
    """Bass program for `cap` slots/core; slots [0,ksplit) read Q-batch 0,
    slots [ksplit,cap) read Q-batch 1."""
    import concourse.bass as bass  # noqa: F401
    import concourse.tile as tile
    from concourse import bacc, mybir

    f32 = mybir.dt.float32
    bf16 = mybir.dt.bfloat16
    AF = mybir.ActivationFunctionType
    ALU = mybir.AluOpType

    nc = bacc.Bacc("TRN2", target_bir_lowering=False, debug=False,
                   num_devices=NCORES)

    # One packed bf16 input blob per core (single DMA; every operand is a
    # flat column-slice view, f32 scale columns via bitcast):
    #   [0:512)        wqs: lhsT (fh,c) at fh*256+c*128   ((wb/2)Wq, wb*Wq)
    #   [512:1024)     wks: same for Wk
    #   [1024:1032)    wvp (128,4) f32 = ALPHA*MFAC*Wv    (bitcast)
    #   [1032:1040)    wvn (128,4) f32 = -CCOEF*wvp       (bitcast)
    #   [1040:1042)    f32 zero column (activation bias AP; avoids the
    #                  framework const-AP TENSOR_LOAD preamble)
    #   [1056+qb*512)  qt[qb]: rhs chunk c at +c*256      (2 batches)
    #   [2080+s*520)   kv[s]: kt chunk c at +c*128, ve at +256 (520 wide)
    QT_OFF = 1056
    KV_OFF = QT_OFF + 2 * 512
    TOT = KV_OFF + 520 * cap
    blob = nc.dram_tensor("blob", [128, TOT], bf16, kind="ExternalInput")
    out = nc.dram_tensor("out", [2, 128, 2, VE_W], bf16,
                         kind="ExternalOutput")

    npairs = -(-cap // 2)

    with tile.TileContext(nc) as tc:
        with ExitStack() as ctx:
            blob_p = ctx.enter_context(tc.tile_pool(name="blobp", bufs=1))
            qfp = ctx.enter_context(tc.tile_pool(name="qfp", bufs=2))
            kfp = ctx.enter_context(tc.tile_pool(name="kfp", bufs=3))
            exq = ctx.enter_context(tc.tile_pool(name="exq", bufs=2))
            osb_p = ctx.enter_context(tc.tile_pool(name="osbp", bufs=2))
            # PSUM budget (8 banks): big 2x2 (qp then o_acc, same tag) +
            # kp 2 + sc 2
            ps_big = ctx.enter_context(
                tc.tile_pool(name="psbig", bufs=2, space="PSUM"))
            ps_kp = ctx.enter_context(
                tc.tile_pool(name="pskp", bufs=2, space="PSUM"))
            ps_sc = ctx.enter_context(
                tc.tile_pool(name="pssc", bufs=min(npairs, 2), space="PSUM"))

            bsb = blob_p.tile([128, TOT], bf16)
            # spread the input DMA issue across three idle sequencers so
            # transfers start during the framework preamble (measured
            # best cut: consts+qt on SP, kv halves on ACT and GPSIMD —
            # slicing qt0/qt1 apart delays the kv pieces and loses ~2us)
            kv_mid = KV_OFF + 520 * ((cap + 1) // 2)
            nc.sync.dma_start(out=bsb[:, :KV_OFF], in_=blob[:, :KV_OFF])
            nc.scalar.dma_start(out=bsb[:, KV_OFF:kv_mid],
                                in_=blob[:, KV_OFF:kv_mid])
            nc.gpsimd.dma_start(out=bsb[:, kv_mid:], in_=blob[:, kv_mid:])

            def wq_l(fh, c):
                o = fh * 256 + c * 128
                return bsb[:, o:o + 128]

            def wk_l(fh, c):
                o = 512 + fh * 256 + c * 128
                return bsb[:, o:o + 128]

            wvp_ap = bsb[:, 1024:1032].bitcast(f32)   # (128,4)
            wvn_ap = bsb[:, 1032:1040].bitcast(f32)
            zero_ap = bsb[:, 1040:1042].bitcast(f32)  # (128,1)

            def qt_ch(qb, c):
                o = QT_OFF + qb * 512 + c * 256
                return bsb[:, o:o + 256]

            def kt_ch(s, c):
                o = KV_OFF + s * 520 + c * 128
                return bsb[:, o:o + 128]

            def ve_ap(s):
                o = KV_OFF + s * 520 + 256
                return bsb[:, o:o + VE_W]

            def ladder_steps(S, pool, ns, n, pfx, scaled, res):
                """Doubling-ladder feature ops as a list of closures so
                independent instances can be emitted interleaved (keeps the
                DVE FIFO free of back-to-back dependent ops).  S is
                (128, ns, 2, n): ns batched instances (slot pairs on the K
                side).  Fills res[:] = [sins, coss] per instance at the end;
                each entry is a list of (128, n) APs indexed [inst][m].

                The K side (scaled=False) returns raw sins and coses, except
                cos8 which carries the last wvp/wvn score scale (it is only a
                pass operand, so its Q partner uses the raw s8 — one fewer
                scale op).  The Q side (scaled=True) returns wv-scaled pass
                operands, with cos ops fused from the f tiles."""
                t = {}

                def tt(name, in0f, in1f):
                    def go():
                        t[name] = pool.tile([128, ns, n], bf16,
                                            tag=f"{pfx}{name}", name=name)
                        nc.vector.tensor_tensor(out=t[name][:], in0=in0f(),
                                                in1=in1f(), op=ALU.mult)
                    return go

                def aff(name, inf, s1, s2):
                    def go():
                        t[name] = pool.tile([128, ns, n], bf16,
                                            tag=f"{pfx}{name}", name=name)
                        nc.vector.tensor_scalar(
                            out=t[name][:], in0=inf(), scalar1=s1,
                            scalar2=s2, op0=ALU.mult, op1=ALU.add)
                    return go

                def F12():
                    t['F12'] = pool.tile([128, ns, 2, n], bf16,
                                         tag=f"{pfx}F12", name="F12")
                    nc.vector.tensor_tensor(out=t['F12'][:], in0=S[:],
                                            in1=S[:], op=ALU.mult)

                def C12():
                    t['C12'] = pool.tile([128, ns, 2, n], bf16,
                                         tag=f"{pfx}C12", name="C12")
                    nc.vector.tensor_scalar(out=t['C12'][:], in0=t['F12'][:],
                                            scalar1=-2.0, scalar2=1.0,
                                            op0=ALU.mult, op1=ALU.add)

                # step order surfaces f8 early: it feeds the last-gating
                # score scale (cs2 on ACT / the K-side scaled cos8), so the
                # m=2 passes unblock ~2 DVE ops sooner
                steps = [
                    F12, C12,
                    tt('s2', lambda: S[:, :, 1, :],
                       lambda: t['C12'][:, :, 0, :]),
                    tt('s4', lambda: t['s2'][:],
                       lambda: t['C12'][:, :, 1, :]),
                    tt('f8', lambda: t['s4'][:], lambda: t['s4'][:]),
                    tt('f4', lambda: t['s2'][:], lambda: t['s2'][:]),
                    aff('cos4', lambda: t['f4'][:], -8.0, 1.0),
                    tt('s8', lambda: t['s4'][:], lambda: t['cos4'][:]),
                ]

                def finish_raw():
                    res[:] = [
                        ([S[:, i, 1, :], t['s4'][:, i, :],
                          t['s8'][:, i, :]],
                         [t['C12'][:, i, 0, :],
                          t['cos4'][:, i, :], t['cos8'][:, i, :]])
                        for i in range(ns)]

                def finish_scaled():
                    res[:] = [([t['ss0'][:, i, :], t['ss1'][:, i, :],
                                t['s8'][:, i, :]],
                               [t[f'cs{m}'][:, i, :] for m in range(NFREQ)])
                              for i in range(ns)]

                if not scaled:
                    # cos8 carries the m=3 score scale (see docstring)
                    steps.append(aff('cos8', lambda: t['f8'][:],
                                     wvn_ap[:, NFREQ - 1:NFREQ],
                                     wvp_ap[:, NFREQ - 1:NFREQ]))
                    steps.append(finish_raw)
                    return steps, []

                # Q-side score scales run on ACT (Identity with per-
                # partition scale/bias APs): they overlap the K ladders on
                # DVE, which stay on the critical path to the scores.
                raws = [lambda: S[:, :, 1, :], lambda: t['s4'][:]]
                fts = [lambda: t['F12'][:, :, 0, :],
                       lambda: t['f4'][:], lambda: t['f8'][:]]
                def mk_ss(m):
                    def go():
                        nm = f'ss{m}'
                        t[nm] = pool.tile([128, ns, n], bf16,
                                          tag=f"{pfx}{nm}", name=nm)
                        nc.scalar.activation(
                            out=t[nm][:], in_=raws[m](), func=AF.Identity,
                            scale=wvp_ap[:, m:m + 1])
                    return go

                def mk_cs(m):
                    def go():
                        nm = f'cs{m}'
                        t[nm] = pool.tile([128, ns, n], bf16,
                                          tag=f"{pfx}{nm}", name=nm)
                        nc.scalar.activation(
                            out=t[nm][:], in_=fts[m](), func=AF.Identity,
                            scale=wvn_ap[:, m:m + 1],
                            bias=wvp_ap[:, m:m + 1])
                    return go

                # dependency-ready order on the ACT FIFO: ss0 needs only
                # the Sin output, cs0/cs1 need f_h/f4 (ladder steps 1/4),
                # ss1 needs s4 (step 6), cs2 needs f8 (step 7)
                scale_steps = [mk_ss(0), mk_cs(0), mk_cs(1), mk_ss(1),
                               mk_cs(2), finish_scaled]
                return steps, scale_steps

            def emit_interleaved(step_lists):
                nmax = max(len(sl) for sl in step_lists)
                for i in range(nmax):
                    for sl in step_lists:
                        if i < len(sl):
                            sl[i]()

            # ---- projections + Sins for everything first (keeps ACT's --
            # in-order FIFO free: Sins never queue behind other work) -----
            qp_ps = ps_big.tile([128, 2, 2, 256], f32, tag="big")
            for qb in range(2):
                for fh in range(2):       # 0: wb/2, 1: wb
                    for c in range(2):
                        nc.tensor.matmul(qp_ps[:, qb, fh, :], wq_l(fh, c),
                                         qt_ch(qb, c),
                                         start=(c == 0), stop=(c == 1))
            SQ = qfp.tile([128, 2, 2, 256], bf16, tag="SQ")
            for qb in range(2):
                nc.scalar.activation(out=SQ[:, qb, :, :],
                                     in_=qp_ps[:, qb, :, :], func=AF.Sin)
            qres = []
            qsteps, qscale_steps = ladder_steps(SQ, qfp, 2, 256, "q", True,
                                                qres)

            pair_slots = [[s for s in (2 * p, 2 * p + 1) if s < cap]
                          for p in range(npairs)]
            ksteps, kres = [], []
            for p in range(npairs):
                slots = pair_slots[p]
                ns = len(slots)
                kp_ps = ps_kp.tile([128, 2, 2, 128], f32, tag="kp")
                for si, s in enumerate(slots):
                    for fh in range(2):
                        for c in range(2):
                            nc.tensor.matmul(kp_ps[:, si, fh, :],
                                             wk_l(fh, c), kt_ch(s, c),
                                             start=(c == 0), stop=(c == 1))
                SK = kfp.tile([128, 2, 2, 128], bf16, tag="SK")
                nc.scalar.activation(out=SK[:, :ns, :, :],
                                     in_=kp_ps[:, :ns, :, :], func=AF.Sin)
                kres.append([])
                ksteps.append(ladder_steps(SK[:, :ns, :, :], kfp, ns,
                                           128, "k", False, kres[p])[0])

            # ---- Q ladder (DVE) then Q scales (ACT, overlaps K ladders)
            for step in qsteps:
                step()
            for step in qscale_steps:
                step()
            qfeat = qres

            # per-qb output accumulators: V partials accumulate in PSUM
            # across a core's slots of that batch (start on first, stop on
            # last) -> only 2 copies + 2 output DMAs at the very end
            qb_slots = [[s for s in range(cap) if (s < ksplit) == (qb == 0)]
                        for qb in range(2)]
            o_acc = [None, None]

            def flush_slot(t):
                p, t_i = t // 2, t % 2
                exp_sb = exq.tile([128, 1, 256], bf16, tag="exp")
                nc.scalar.activation(out=exp_sb[:, 0, :],
                                     in_=sc_tiles[p][:, t_i, :],
                                     func=AF.Exp)
                qb = 0 if t < ksplit else 1
                if o_acc[qb] is None:
                    o_acc[qb] = ps_big.tile([128, 2, 512], f32,
                                            tag="big", name=f"oacc{qb}")
                first = (t == qb_slots[qb][0])
                lasts = (t == qb_slots[qb][-1])
                for ic in range(2):
                    nc.tensor.matmul(
                        o_acc[qb][:, ic, :VE_W],
                        exp_sb[:, 0, ic * 128:(ic + 1) * 128],
                        ve_ap(t),
                        start=first, stop=lasts,
                        skip_group_check=True)

            # ---- per pair: K ladder (DVE) -> scores (PE); flushes are
            # staggered one SLOT behind the score stream so the PE FIFO
            # never stalls score matmuls behind V-aggregations ------------
            # Small dep-free 64-col dummy matmuls are sprinkled between the
            # early (scale-gated) score passes: they execute only during
            # otherwise-idle stalls and keep PE's 2.4GHz p-state from
            # resetting (idle drops it to 1.2GHz, doubling every matmul).
            warm_ps = ps_kp.tile([128, 2, 2, 128], f32, tag="kp",
                                 name="warm")

            def warm(k):
                for _ in range(k):
                    nc.tensor.matmul(warm_ps[:, 0, 0, :64], wq_l(0, 0),
                                     wq_l(1, 0)[:, :64], start=True,
                                     stop=True, skip_group_check=True)

            sc_tiles = []
            done = 0     # slots whose scores are fully emitted
            flushed = 0
            for p in range(npairs):
                slots = pair_slots[p]
                for step in ksteps[p]:
                    step()
                sc_ps = ps_sc.tile([128, 2, 256], f32, tag="sc")
                sc_tiles.append(sc_ps)
                for si, s in enumerate(slots):
                    qb = 0 if s < ksplit else 1
                    qsins, qcoss = qfeat[qb]
                    ksins, kcoss = kres[p][si]
                    for m in range(NFREQ):
                        nc.tensor.matmul(sc_ps[:, si, :], kcoss[m],
                                         qsins[m],
                                         start=(m == 0), stop=False)
                        nc.tensor.matmul(sc_ps[:, si, :], ksins[m],
                                         qcoss[m],
                                         start=False, stop=(m == NFREQ - 1))
                        if si == 0 and m < NFREQ - 1:
                            warm(4)
                    done = s + 1
                    while flushed < done - 1:
                        flush_slot(flushed)
                        flushed += 1
            while flushed < cap:
                flush_slot(flushed)
                flushed += 1

            # ---- final copies + output DMAs ----------------------------
            # qb0's accumulators close first and DVE is idle by then, so
            # qb0 copies ride DVE; qb1 splits per-ic across ACT (after its
            # last exp) and DVE so the two halves run in parallel.
            for qb in range(2):
                if o_acc[qb] is None:
                    continue
                o_sb = osb_p.tile([128, 2, VE_W], bf16, tag="osb")
                for ic in range(2):
                    eng = (nc.scalar.copy if (qb == 1 and ic == 0)
                           else nc.vector.tensor_copy)
                    eng(o_sb[:, ic, :], o_acc[qb][:, ic, :VE_W])
                nc.sync.dma_start(out=out[qb], in_=o_sb[:])

    nc.compile()
    return nc


def _get_program(cap: int, ksplit: int):
    key = (cap, ksplit, tuple(sorted(CONFIG.items())))
    if key not in _prog_cache:
        _prog_cache[key] = _build_program(cap, ksplit)
    return _prog_cache[key]


def _chunkT(a2d: np.ndarray, nfree: int) -> np.ndarray:
    """(n, 256) row-major -> (128, 2, n): [p, c, n] = a2d[n, 128c + p]."""
    return np.ascontiguousarray(
        a2d.T.reshape(2, 128, nfree).transpose(1, 0, 2))


def _schedule(slot_lists):
    """Pack slots into NCORES cores x (groupA: ksplit of one batch, groupB:
    cap-ksplit of one batch).  Dummy padding allowed.  Returns (cap, ksplit,
    cores) with cores[c] = (batchA, slotsA, batchB, slotsB)."""
    total = sum(len(v) for v in slot_lists.values())
    batches = [b for b, v in slot_lists.items() if len(v) > 0]
    for cap in range(max(1, -(-total // NCORES)), NKV // SLOT_KEYS + 1):
        for k in range(cap, (cap - 1) // 2, -1):
            g1, g2 = k, cap - k
            # per-batch options: (nA_groups, nB_groups) covering its count
            opts = []
            for b in batches:
                c = len(slot_lists[b])
                o = []
                for a in range(0, min(NCORES, -(-c // g1)) + 1):
                    rem = c - a * g1
                    if g2 > 0:
                        nb = max(0, -(-rem // g2))
                    else:
                        if rem > 0:
                            continue
                        nb = 0
                    if nb > NCORES:
                        continue
                    o.append((a, nb))
                opts.append(o)
            # exact DP over (A_groups_used, B_groups_used)
            chains = {(0, 0): []}
            for o in opts:
                nxt = {}
                for st, ch in chains.items():
                    for (a, nb) in o:
                        s2 = (st[0] + a, st[1] + nb)
                        if s2[0] <= NCORES and s2[1] <= NCORES \
                                and s2 not in nxt:
                            nxt[s2] = ch + [(a, nb)]
                chains = nxt
                if not chains:
                    break
            if not chains:
                continue
            choice = next(iter(chains.values()))
            achunks, bchunks = [], []
            for bi, b in enumerate(batches):
                a, nb = choice[bi]
                slots = slot_lists[b]
                pos = 0
                for _ in range(a):
                    achunks.append((b, slots[pos:pos + g1]))
                    pos += g1
                for _ in range(nb):
                    bchunks.append((b, slots[pos:pos + g2]))
                    pos += g2
            achunks += [(None, [])] * (NCORES - len(achunks))
            bchunks += [(None, [])] * (NCORES - len(bchunks))
            cores = [(achunks[c][0], achunks[c][1],
                      bchunks[c][0], bchunks[c][1])
                     for c in range(NCORES)]
            return cap, k, cores
    raise RuntimeError("schedule failed")


def _prepare(Q_batch, K_batch, V_batch, valid_lens, Wq, Wk, Wv):
    import ml_dtypes
    bfd = ml_dtypes.bfloat16

    Q = np.asarray(Q_batch, np.float32)
    K = np.asarray(K_batch, np.float32)
    V = np.asarray(V_batch, np.float32)
    L = np.asarray(valid_lens).astype(np.int64)
    Wq = np.asarray(Wq, np.float32)
    Wk = np.asarray(Wk, np.float32)
    Wv = np.asarray(Wv, np.float32)

    slot_lists = {}
    for b in range(B):
        nblk = min(max(1, int(-(-int(L[b]) // SLOT_KEYS))), NKV // SLOT_KEYS)
        slot_lists[b] = [(b, blk * SLOT_KEYS) for blk in range(nblk)]
    cap, ksplit, cores = _schedule(slot_lists)

    wqt = _chunkT(Wq, 128)
    wkt = _chunkT(Wk, 128)
    # flat (128, 512): [fh, c] stationary at fh*256 + c*128
    wqs = np.stack([0.5 * WB * wqt, WB * wqt],
                   axis=1).astype(bfd).reshape(128, 512)
    wks = np.stack([0.5 * WB * wkt, WB * wkt],
                   axis=1).astype(bfd).reshape(128, 512)
    al = np.asarray(ALPHA, np.float32) * np.asarray(MFAC, np.float32)
    wvp = np.zeros((128, 4), np.float32)
    wvp[:, :NFREQ] = al[None, :] * Wv[:, None]
    wvn = np.zeros((128, 4), np.float32)
    wvn[:, :NFREQ] = -np.asarray(CCOEF, np.float32)[None, :] \
        * wvp[:, :NFREQ]
    wvp16 = np.ascontiguousarray(wvp).view(np.uint16).view(bfd)  # (128,8)
    wvn16 = np.ascontiguousarray(wvn).view(np.uint16).view(bfd)

    qts = {b: _chunkT(Q[b], 256).astype(bfd).reshape(128, 512)
           for b in range(B)}

    QT_OFF = 1056
    KV_OFF = QT_OFF + 2 * 512
    TOT = KV_OFF + 520 * cap

    in_maps, core_qbs = [], []
    for (ba, sa, bb, sb) in cores:
        slots = list(sa) + [None] * (ksplit - len(sa)) \
            + list(sb) + [None] * ((cap - ksplit) - len(sb))
        core_qbs.append((ba, bb))
        blob = np.zeros((128, TOT), bfd)
        blob[:, 0:512] = wqs
        blob[:, 512:1024] = wks
        blob[:, 1024:1032] = wvp16
        blob[:, 1032:1040] = wvn16
        if ba is not None:
            blob[:, QT_OFF:QT_OFF + 512] = qts[ba]
        if bb is not None:
            blob[:, QT_OFF + 512:QT_OFF + 1024] = qts[bb]
        for si, it in enumerate(slots):
            if it is None:
                continue
            b, j0 = it
            o = KV_OFF + si * 520
            blob[:, o:o + 256] = _chunkT(
                K[b, j0:j0 + SLOT_KEYS],
                SLOT_KEYS).astype(bfd).reshape(128, 256)
            nval = int(np.clip(int(L[b]) - j0, 0, SLOT_KEYS))
            vv = np.zeros((128, VE_W), np.float32)
            vv[:nval, :256] = V[b, j0:j0 + nval]
            vv[:nval, DEN_COL] = 1.0
            blob[:, o + 256:o + 520] = vv.astype(bfd)
        in_maps.append({"blob": blob})
    return cap, ksplit, core_qbs, in_maps


def _gather(core_qbs, results) -> np.ndarray:
    acc = np.zeros((B, NQ, 257), np.float64)
    for c, (ba, bb) in enumerate(core_qbs):
        o = np.asarray(results[c]["out"], np.float64)  # (2,128,2,VE_W)
        for qb, b in ((0, ba), (1, bb)):
            if b is None:
                continue
            # partial[i = ic*128 + p] = o[qb][p, ic]
            acc[b] += o[qb].transpose(1, 0, 2).reshape(NQ, VE_W)[:, :257]
    return (acc[:, :, :256] / acc[:, :, 256:257]).astype(np.float32)


def _install_ntff_hook():
    """Register the axon NTFF profile hook that bass_utils reads via
    antenv.axon_hooks (the shipped antenv stub lacks that module)."""
    import contextlib
    import ctypes
    import sys
    import types

    try:
        from antenv.axon_hooks import get_axon_ntff_profile_hook
        if get_axon_ntff_profile_hook() is not None:
            return
    except ImportError:
        pass

    so_path = "/opt/axon/libaxon_pjrt.so"
    if not os.path.exists(so_path):
        return
    lib = ctypes.CDLL(so_path)
    if not hasattr(lib, "axon_start_nrt_profile"):
        return
    lib.axon_start_nrt_profile.argtypes = [
        ctypes.POINTER(ctypes.c_int64), ctypes.c_size_t]
    lib.axon_start_nrt_profile.restype = ctypes.c_int64
    lib.axon_stop_nrt_profile.argtypes = [ctypes.c_char_p]
    lib.axon_stop_nrt_profile.restype = ctypes.c_int64

    @contextlib.contextmanager
    def _hook(output_dir, device_ids):
        import jax
        jax.devices()
        if device_ids:
            ids = (ctypes.c_int64 * len(device_ids))(*device_ids)
            rc = lib.axon_start_nrt_profile(ids, len(device_ids))
        else:
            rc = lib.axon_start_nrt_profile(None, 0)
        if rc != 0:
            raise RuntimeError(f"axon_start_nrt_profile rc={rc}")
        try:
            yield
        finally:
            n = lib.axon_stop_nrt_profile(str(output_dir).encode())
            print(f"ntff profile: {n} file(s) written to {output_dir}")

    mod = types.ModuleType("antenv.axon_hooks")
    mod.get_axon_ntff_profile_hook = lambda: _hook
    mod.set_axon_ntff_profile_hook = lambda h: None
    sys.modules["antenv.axon_hooks"] = mod
    import antenv
    antenv.axon_hooks = mod


def run(Q_batch, K_batch, V_batch, valid_lens, Wq, Wk, Wv,
        trace: bool = False):
    """Returns (output, exec_time_ns_or_None)."""
    from concourse.bass_utils import run_bass_kernel_spmd

    if trace:
        _install_ntff_hook()

    cap, ksplit, core_qbs, in_maps = _prepare(
        Q_batch, K_batch, V_batch, valid_lens, Wq, Wk, Wv)
    nc = _get_program(cap, ksplit)

    if os.environ.get("ADD_ATTN_SIM"):
        from concourse.bass_interp import CoreSim
        ncores = int(os.environ.get("ADD_ATTN_SIM_CORES", NCORES))
        results = []
        for c in range(ncores):
            sim = CoreSim(nc)
            for name, arr in in_maps[c].items():
                sim.tensor(name)[:] = arr
            sim.simulate()
            results.append({"out": np.array(sim.tensor("out"))})
        return _gather(core_qbs[:ncores], results), None

    res = run_bass_kernel_spmd(nc, in_maps, core_ids=list(range(NCORES)),
                               trace=trace)
    return _gather(core_qbs, res.results), res.exec_time_ns


def kernel(Q_batch, K_batch, V_batch, valid_lens, Wq, Wk, Wv):
    out, _ = run(Q_batch, K_batch, V_batch, valid_lens, Wq, Wk, Wv)
    return out


# revision 57
# speedup vs baseline: 1.0138x; 1.0138x over previous
"""Additive (Bahdanau) attention on TRN2 via a separable sine expansion, SPMD x8.

Math per batch b (Q (256,256), K (1024,256), V (1024,256), H=128):
    qp = Q @ Wq.T; kp = K @ Wk.T
    s[i,j] = sum_h Wv[h] * tanh(qp[i,h] + kp[j,h])
    out    = softmax_j(s, j < valid_len) @ V

The baseline materialized qp[i,h]+kp[j,h] on DVE (one tensor_scalar_add per
key, ~277 ns each -> ~145 us).  This kernel instead approximates tanh with a
3-harmonic sine series (offline weighted LSQ on the input measure; m=2 is
kept only as an angle-doubling intermediate -- its score passes and scales
are dropped):

    tanh(x) ~= sum_m alpha_m sin(m*wb*x),  m in {1,4,8},  wb ~ 0.279

Since sin(w(a+b)) = sin(wa)cos(wb) + cos(wa)sin(wb), each harmonic becomes
TWO matmul passes over per-side features, putting the O(NQ*NKV) work on the
otherwise-idle PE instead of DVE:

    s[i,j] ~= sum_m sum_h [alpha_m Wv[h] sin_m(qp)] cos_m(kp) + (sin<->cos)

ACT's Sin table only admits [-pi,pi] inputs, so only the base harmonic is
evaluated directly (|wb*qp| <= 1.31) and the rest come from angle-doubling
products on DVE (all bf16 SBUF):

    s1 = Sin(wb x)           [ACT, batched with s_h = Sin(wb x / 2)]
    cos1 = 1-2*s_h^2         sin2 = 2 s1 cos1 = 2*s2,  cos2 = 1-2*s1^2
    sin4 = 4*(s2 cos2),      cos4 = 1-8*s2^2
    sin8 = 8*(s4 cos4),      cos8 = 1-32*s4^2

The m-factors and doubling constants fold into the alpha/Wv score-pass scales
(tensor_scalar with per-partition [128,1] columns), so each feature is one
DVE op.  tanh's exp-free basis avoids ACT table swaps except one Sin->Exp
switch (all Sins are emitted before all Exps).

Work decomposition: "slots" of 128 contiguous keys of one batch, load-balanced
over the 8 cores like the baseline (unnormalized softmax partials summed on
the host in f64; masked keys zeroed via the [V|1] matrix).  Q-side features
are computed once per distinct batch on a core: slots are scheduled so each
core serves at most 2 batches with a uniform [qb0]*ksplit + [qb1]*(cap-ksplit)
pattern (dummy all-zero slots pad infeasible splits; they contribute nothing).

Pipeline engineering (see git-of-thought in comments):
  * all inputs ride in ONE packed SBUF blob filled by 3 DMAs issued from
    3 different sequencers (SP/ACT/GPSIMD) so transfers overlap the
    framework preamble; operands are flat column slices, f32 scale
    columns via bitcast
  * per-core V partials accumulate in PSUM across slots per Q-batch
    (only 2 PSUM->SBUF copies + 2 output DMAs at the end); the o_acc
    banks reuse the retired qp projection banks via a shared pool tag
  * emission order keeps every engine FIFO hazard-free: all Sins before
    all Exps (one ACT table switch), score matmuls never queue behind
    V-aggregations, Q-side score scales run on ACT while the K ladders
    run on DVE

Measured on TRN2 (axon), seed-0 inputs (cap=4, ksplit=3): 31.2-35us HW
exec (device-load dependent; baseline 145-174us), rel err 5.9e-3.
"""

import os
from contextlib import ExitStack

import numpy as np

B, NQ, NKV, D, H = 8, 256, 1024, 256, 128
NCORES = 8
SLOT_KEYS = 128
VE_W = 264               # 256 V cols + 1 ones col + 7 pad
DEN_COL = 256

# sine-ladder parameters (offline fit to seed-0 input statistics).
# Harmonics {1,4,8}*wb: m=2 is only an angle-doubling intermediate -- its
# score passes and scale ops are dropped (alpha_2 was 0.14; wRMS 1.40e-2
# vs 1.24e-2 with it, end-to-end ~7e-3, still 2.8x under the 2e-2 gate).
WB = 0.2789
ALPHA = [1.4149, 0.3533, 0.0771]  # m = 1,4,8
MFAC = [1.0, 4.0, 8.0]            # sin_m = MFAC*s_m  (s_m = ladder feature)
CCOEF = [2.0, 8.0, 32.0]          # cos_m = 1 - CCOEF*f_m
NFREQ = 3

CONFIG: dict = {}

_prog_cache: dict[tuple, object] = {}


def _build_program(cap: int, ksplit: int):
    """Bass program for `cap` slots/core; slots [0,ksplit) read Q-batch 0,
    slots [ksplit,cap) read Q-batch 1."""
    import concourse.bass as bass  # noqa: F401
    import concourse.tile as tile
    from concourse import bacc, mybir

    f32 = mybir.dt.float32
    bf16 = mybir.dt.bfloat16
    AF = mybir.ActivationFunctionType
    ALU = mybir.AluOpType

    nc = bacc.Bacc("TRN2", target_bir_lowering=False, debug=False,
                   num_devices=NCORES)

    # One packed bf16 input blob per core (single DMA; every operand is a
    # flat column-slice view, f32 scale columns via bitcast):
    #   [0:512)        wqs: lhsT (fh,c) at fh*256+c*128   ((wb/2)Wq, wb*Wq)
    #   [512:1024)     wks: same for Wk
    #   [1024:1032)    wvp (128,4) f32 = ALPHA*MFAC*Wv    (bitcast)
    #   [1032:1040)    wvn (128,4) f32 = -CCOEF*wvp       (bitcast)
    #   [1040:1042)    f32 zero column (activation bias AP; avoids the
    #                  framework const-AP TENSOR_LOAD preamble)
    #   [1056+qb*512)  qt[qb]: rhs chunk c at +c*256      (2 batches)
    #   [2080+s*520)   kv[s]: kt chunk c at +c*128, ve at +256 (520 wide)
    QT_OFF = 1056
    KV_OFF = QT_OFF + 2 * 512
    TOT = KV_OFF + 520 * cap
    blob = nc.dram_tensor("blob", [128, TOT], bf16, kind="ExternalInput")
    out = nc.dram_tensor("out", [2, 128, 2, VE_W], bf16,
                         kind="ExternalOutput")

    npairs = -(-cap // 2)

    with tile.TileContext(nc) as tc:
        with ExitStack() as ctx:
            blob_p = ctx.enter_context(tc.tile_pool(name="blobp", bufs=1))
            qfp = ctx.enter_context(tc.tile_pool(name="qfp", bufs=2))
            kfp = ctx.enter_context(tc.tile_pool(name="kfp", bufs=3))
            exq = ctx.enter_context(tc.tile_pool(name="exq", bufs=2))
            osb_p = ctx.enter_context(tc.tile_pool(name="osbp", bufs=2))
            # PSUM budget (8 banks): big 2x2 (qp then o_acc, same tag) +
            # kp 2 + sc 2
            ps_big = ctx.enter_context(
                tc.tile_pool(name="psbig", bufs=2, space="PSUM"))
            ps_kp = ctx.enter_context(
                tc.tile_pool(name="pskp", bufs=2, space="PSUM"))
            ps_sc = ctx.enter_context(
                tc.tile_pool(name="pssc", bufs=min(npairs, 2), space="PSUM"))

            bsb = blob_p.tile([128, TOT], bf16)
            # spread the input DMA issue across three idle sequencers so
            # transfers start during the framework preamble (measured
            # best cut: consts+qt on SP, kv halves on ACT and GPSIMD —
            # slicing qt0/qt1 apart delays the kv pieces and loses ~2us)
            # the SP piece is cut once more at qt0|qt1: queue FIFOs drain
            # the first sub-piece earlier, so qb0's projection->Sin chain
            # starts ~0.6us sooner (kv pieces stay as-is — moving them
            # later measurably loses)
            kv_mid = KV_OFF + 520 * ((cap + 1) // 2)
            qcut = QT_OFF + 512
            nc.sync.dma_start(out=bsb[:, :qcut], in_=blob[:, :qcut])
            nc.sync.dma_start(out=bsb[:, qcut:KV_OFF],
                              in_=blob[:, qcut:KV_OFF])
            nc.scalar.dma_start(out=bsb[:, KV_OFF:kv_mid],
                                in_=blob[:, KV_OFF:kv_mid])
            nc.gpsimd.dma_start(out=bsb[:, kv_mid:], in_=blob[:, kv_mid:])

            def wq_l(fh, c):
                o = fh * 256 + c * 128
                return bsb[:, o:o + 128]

            def wk_l(fh, c):
                o = 512 + fh * 256 + c * 128
                return bsb[:, o:o + 128]

            wvp_ap = bsb[:, 1024:1032].bitcast(f32)   # (128,4)
            wvn_ap = bsb[:, 1032:1040].bitcast(f32)
            zero_ap = bsb[:, 1040:1042].bitcast(f32)  # (128,1)

            def qt_ch(qb, c):
                o = QT_OFF + qb * 512 + c * 256
                return bsb[:, o:o + 256]

            def kt_ch(s, c):
                o = KV_OFF + s * 520 + c * 128
                return bsb[:, o:o + 128]

            def ve_ap(s):
                o = KV_OFF + s * 520 + 256
                return bsb[:, o:o + VE_W]

            def ladder_steps(S, pool, ns, n, pfx, scaled, res):
                """Doubling-ladder feature ops as a list of closures so
                independent instances can be emitted interleaved (keeps the
                DVE FIFO free of back-to-back dependent ops).  S is
                (128, ns, 2, n): ns batched instances (slot pairs on the K
                side).  Fills res[:] = [sins, coss] per instance at the end;
                each entry is a list of (128, n) APs indexed [inst][m].

                The K side (scaled=False) returns raw sins and coses, except
                cos8 which carries the last wvp/wvn score scale (it is only a
                pass operand, so its Q partner uses the raw s8 — one fewer
                scale op).  The Q side (scaled=True) returns wv-scaled pass
                operands, with cos ops fused from the f tiles."""
                t = {}

                def tt(name, in0f, in1f):
                    def go():
                        t[name] = pool.tile([128, ns, n], bf16,
                                            tag=f"{pfx}{name}", name=name)
                        nc.vector.tensor_tensor(out=t[name][:], in0=in0f(),
                                                in1=in1f(), op=ALU.mult)
                    return go

                def aff(name, inf, s1, s2):
                    def go():
                        t[name] = pool.tile([128, ns, n], bf16,
                                            tag=f"{pfx}{name}", name=name)
                        nc.vector.tensor_scalar(
                            out=t[name][:], in0=inf(), scalar1=s1,
                            scalar2=s2, op0=ALU.mult, op1=ALU.add)
                    return go

                def F12():
                    t['F12'] = pool.tile([128, ns, 2, n], bf16,
                                         tag=f"{pfx}F12", name="F12")
                    nc.vector.tensor_tensor(out=t['F12'][:], in0=S[:],
                                            in1=S[:], op=ALU.mult)

                def C12():
                    t['C12'] = pool.tile([128, ns, 2, n], bf16,
                                         tag=f"{pfx}C12", name="C12")
                    nc.vector.tensor_scalar(out=t['C12'][:], in0=t['F12'][:],
                                            scalar1=-2.0, scalar2=1.0,
                                            op0=ALU.mult, op1=ALU.add)

                # step order surfaces f8 early: it feeds the last-gating
                # score scale (cs2 on ACT / the K-side scaled cos8), so the
                # m=2 passes unblock ~2 DVE ops sooner
                steps = [
                    F12, C12,
                    tt('s2', lambda: S[:, :, 1, :],
                       lambda: t['C12'][:, :, 0, :]),
                    tt('s4', lambda: t['s2'][:],
                       lambda: t['C12'][:, :, 1, :]),
                    tt('f8', lambda: t['s4'][:], lambda: t['s4'][:]),
                    tt('f4', lambda: t['s2'][:], lambda: t['s2'][:]),
                    aff('cos4', lambda: t['f4'][:], -8.0, 1.0),
                    tt('s8', lambda: t['s4'][:], lambda: t['cos4'][:]),
                ]

                def finish_raw():
                    res[:] = [
                        ([S[:, i, 1, :], t['s4'][:, i, :],
                          t['s8'][:, i, :]],
                         [t['C12'][:, i, 0, :],
                          t['cos4'][:, i, :], t['cos8'][:, i, :]])
                        for i in range(ns)]

                def finish_scaled():
                    res[:] = [([t['ss0'][:, i, :], t['ss1'][:, i, :],
                                t['s8'][:, i, :]],
                               [t[f'cs{m}'][:, i, :] for m in range(NFREQ)])
                              for i in range(ns)]

                if not scaled:
                    # cos8 carries the m=3 score scale (see docstring)
                    steps.append(aff('cos8', lambda: t['f8'][:],
                                     wvn_ap[:, NFREQ - 1:NFREQ],
                                     wvp_ap[:, NFREQ - 1:NFREQ]))
                    steps.append(finish_raw)
                    return steps, []

                # Q-side score scales run on ACT (Identity with per-
                # partition scale/bias APs): they overlap the K ladders on
                # DVE, which stay on the critical path to the scores.
                raws = [lambda: S[:, :, 1, :], lambda: t['s4'][:]]
                fts = [lambda: t['F12'][:, :, 0, :],
                       lambda: t['f4'][:], lambda: t['f8'][:]]
                def mk_ss(m):
                    def go():
                        nm = f'ss{m}'
                        t[nm] = pool.tile([128, ns, n], bf16,
                                          tag=f"{pfx}{nm}", name=nm)
                        nc.scalar.activation(
                            out=t[nm][:], in_=raws[m](), func=AF.Identity,
                            scale=wvp_ap[:, m:m + 1])
                    return go

                def mk_cs(m):
                    def go():
                        nm = f'cs{m}'
                        t[nm] = pool.tile([128, ns, n], bf16,
                                          tag=f"{pfx}{nm}", name=nm)
                        nc.scalar.activation(
                            out=t[nm][:], in_=fts[m](), func=AF.Identity,
                            scale=wvn_ap[:, m:m + 1],
                            bias=wvp_ap[:, m:m + 1])
                    return go

                # dependency-ready order on the ACT FIFO: ss0 needs only
                # the Sin output, cs0/cs1 need f_h/f4 (ladder steps 1/4),
                # ss1 needs s4 (step 6), cs2 needs f8 (step 7)
                scale_steps = [mk_ss(0), mk_cs(0), mk_cs(1), mk_ss(1),
                               mk_cs(2), finish_scaled]
                return steps, scale_steps

            def emit_interleaved(step_lists):
                nmax = max(len(sl) for sl in step_lists)
                for i in range(nmax):
                    for sl in step_lists:
                        if i < len(sl):
                            sl[i]()

            # ---- projections + Sins for everything first (keeps ACT's --
            # in-order FIFO free: Sins never queue behind other work) -----
            qp_ps = ps_big.tile([128, 2, 2, 256], f32, tag="big")
            for qb in range(2):
                for fh in range(2):       # 0: wb/2, 1: wb
                    for c in range(2):
                        nc.tensor.matmul(qp_ps[:, qb, fh, :], wq_l(fh, c),
                                         qt_ch(qb, c),
                                         start=(c == 0), stop=(c == 1))
            SQ = qfp.tile([128, 2, 2, 256], bf16, tag="SQ")
            for qb in range(2):
                nc.scalar.activation(out=SQ[:, qb, :, :],
                                     in_=qp_ps[:, qb, :, :], func=AF.Sin)
            qres = []
            qsteps, qscale_steps = ladder_steps(SQ, qfp, 2, 256, "q", True,
                                                qres)

            pair_slots = [[s for s in (2 * p, 2 * p + 1) if s < cap]
                          for p in range(npairs)]
            ksteps, kres = [], []
            for p in range(npairs):
                slots = pair_slots[p]
                ns = len(slots)
                kp_ps = ps_kp.tile([128, 2, 2, 128], f32, tag="kp")
                for si, s in enumerate(slots):
                    for fh in range(2):
                        for c in range(2):
                            nc.tensor.matmul(kp_ps[:, si, fh, :],
                                             wk_l(fh, c), kt_ch(s, c),
                                             start=(c == 0), stop=(c == 1))
                SK = kfp.tile([128, 2, 2, 128], bf16, tag="SK")
                nc.scalar.activation(out=SK[:, :ns, :, :],
                                     in_=kp_ps[:, :ns, :, :], func=AF.Sin)
                kres.append([])
                ksteps.append(ladder_steps(SK[:, :ns, :, :], kfp, ns,
                                           128, "k", False, kres[p])[0])

            # ---- Q ladder (DVE) then Q scales (ACT, overlaps K ladders)
            for step in qsteps:
                step()
            for step in qscale_steps:
                step()
            qfeat = qres

            # per-qb output accumulators: V partials accumulate in PSUM
            # across a core's slots of that batch (start on first, stop on
            # last) -> only 2 copies + 2 output DMAs at the very end
            qb_slots = [[s for s in range(cap) if (s < ksplit) == (qb == 0)]
                        for qb in range(2)]
            o_acc = [None, None]

            def flush_slot(t):
                p, t_i = t // 2, t % 2
                exp_sb = exq.tile([128, 1, 256], bf16, tag="exp")
                nc.scalar.activation(out=exp_sb[:, 0, :],
                                     in_=sc_tiles[p][:, t_i, :],
                                     func=AF.Exp)
                qb = 0 if t < ksplit else 1
                if o_acc[qb] is None:
                    o_acc[qb] = ps_big.tile([128, 2, 512], f32,
                                            tag="big", name=f"oacc{qb}")
                first = (t == qb_slots[qb][0])
                lasts = (t == qb_slots[qb][-1])
                for ic in range(2):
                    nc.tensor.matmul(
                        o_acc[qb][:, ic, :VE_W],
                        exp_sb[:, 0, ic * 128:(ic + 1) * 128],
                        ve_ap(t),
                        start=first, stop=lasts,
                        skip_group_check=True)

            # ---- per pair: K ladder (DVE) -> scores (PE); flushes are
            # staggered one SLOT behind the score stream so the PE FIFO
            # never stalls score matmuls behind V-aggregations ------------
            # Small dep-free 64-col dummy matmuls are sprinkled between the
            # early (scale-gated) score passes: they execute only during
            # otherwise-idle stalls and keep PE's 2.4GHz p-state from
            # resetting (idle drops it to 1.2GHz, doubling every matmul).
            warm_ps = ps_kp.tile([128, 2, 2, 128], f32, tag="kp",
                                 name="warm")

            def warm(k):
                for _ in range(k):
                    nc.tensor.matmul(warm_ps[:, 0, 0, :64], wq_l(0, 0),
                                     wq_l(1, 0)[:, :64], start=True,
                                     stop=True, skip_group_check=True)

            sc_tiles = []
            done = 0     # slots whose scores are fully emitted
            flushed = 0
            for p in range(npairs):
                slots = pair_slots[p]
                for step in ksteps[p]:
                    step()
                sc_ps = ps_sc.tile([128, 2, 256], f32, tag="sc")
                sc_tiles.append(sc_ps)
                for si, s in enumerate(slots):
                    qb = 0 if s < ksplit else 1
                    qsins, qcoss = qfeat[qb]
                    ksins, kcoss = kres[p][si]
                    for m in range(NFREQ):
                        nc.tensor.matmul(sc_ps[:, si, :], kcoss[m],
                                         qsins[m],
                                         start=(m == 0), stop=False)
                        nc.tensor.matmul(sc_ps[:, si, :], ksins[m],
                                         qcoss[m],
                                         start=False, stop=(m == NFREQ - 1))
                        if si == 0 and m < NFREQ - 1:
                            warm(4)
                    done = s + 1
                    while flushed < done - 1:
                        flush_slot(flushed)
                        flushed += 1
            while flushed < cap:
                flush_slot(flushed)
                flushed += 1

            # ---- final copies + output DMAs ----------------------------
            # qb0's accumulators close first and DVE is idle by then, so
            # qb0 copies ride DVE; qb1 splits per-ic across ACT (after its
            # last exp) and DVE so the two halves run in parallel.
            for qb in range(2):
                if o_acc[qb] is None:
                    continue
                o_sb = osb_p.tile([128, 2, VE_W], bf16, tag="osb")
                for ic in range(2):
                    eng = (nc.scalar.copy if (qb == 1 and ic == 0)
                           else nc.vector.tensor_copy)
                    eng(o_sb[:, ic, :], o_acc[qb][:, ic, :VE_W])
                nc.sync.dma_start(out=out[qb], in_=o_sb[:])

    nc.compile()
    return nc


def _get_program(cap: int, ksplit: int):
    key = (cap, ksplit, tuple(sorted(CONFIG.items())))
    if key not in _prog_cache:
        _prog_cache[key] = _build_program(cap, ksplit)
    return _prog_cache[key]


def _chunkT(a2d: np.ndarray, nfree: int) -> np.ndarray:
    """(n, 256) row-major -> (128, 2, n): [p, c, n] = a2d[n, 128c + p]."""
    return np.ascontiguousarray(
        a2d.T.reshape(2, 128, nfree).transpose(1, 0, 2))


def _schedule(slot_lists):
    """Pack slots into NCORES cores x (groupA: ksplit of one batch, groupB:
    cap-ksplit of one batch).  Dummy padding allowed.  Returns (cap, ksplit,
    cores) with cores[c] = (batchA, slotsA, batchB, slotsB)."""
    total = sum(len(v) for v in slot_lists.values())
    batches = [b for b, v in slot_lists.items() if len(v) > 0]
    for cap in range(max(1, -(-total // NCORES)), NKV // SLOT_KEYS + 1):
        for k in range(cap, (cap - 1) // 2, -1):
            g1, g2 = k, cap - k
            # per-batch options: (nA_groups, nB_groups) covering its count
            opts = []
            for b in batches:
                c = len(slot_lists[b])
                o = []
                for a in range(0, min(NCORES, -(-c // g1)) + 1):
                    rem = c - a * g1
                    if g2 > 0:
                        nb = max(0, -(-rem // g2))
                    else:
                        if rem > 0:
                            continue
                        nb = 0
                    if nb > NCORES:
                        continue
                    o.append((a, nb))
                opts.append(o)
            # exact DP over (A_groups_used, B_groups_used)
            chains = {(0, 0): []}
            for o in opts:
                nxt = {}
                for st, ch in chains.items():
                    for (a, nb) in o:
                        s2 = (st[0] + a, st[1] + nb)
                        if s2[0] <= NCORES and s2[1] <= NCORES \
                                and s2 not in nxt:
                            nxt[s2] = ch + [(a, nb)]
                chains = nxt
                if not chains:
                    break
            if not chains:
                continue
            choice = next(iter(chains.values()))
            achunks, bchunks = [], []
            for bi, b in enumerate(batches):
                a, nb = choice[bi]
                slots = slot_lists[b]
                pos = 0
                for _ in range(a):
                    achunks.append((b, slots[pos:pos + g1]))
                    pos += g1
                for _ in range(nb):
                    bchunks.append((b, slots[pos:pos + g2]))
                    pos += g2
            achunks += [(None, [])] * (NCORES - len(achunks))
            bchunks += [(None, [])] * (NCORES - len(bchunks))
            cores = [(achunks[c][0], achunks[c][1],
                      bchunks[c][0], bchunks[c][1])
                     for c in range(NCORES)]
            return cap, k, cores
    raise RuntimeError("schedule failed")


def _prepare(Q_batch, K_batch, V_batch, valid_lens, Wq, Wk, Wv):
    import ml_dtypes
    bfd = ml_dtypes.bfloat16

    Q = np.asarray(Q_batch, np.float32)
    K = np.asarray(K_batch, np.float32)
    V = np.asarray(V_batch, np.float32)
    L = np.asarray(valid_lens).astype(np.int64)
    Wq = np.asarray(Wq, np.float32)
    Wk = np.asarray(Wk, np.float32)
    Wv = np.asarray(Wv, np.float32)

    slot_lists = {}
    for b in range(B):
        nblk = min(max(1, int(-(-int(L[b]) // SLOT_KEYS))), NKV // SLOT_KEYS)
        slot_lists[b] = [(b, blk * SLOT_KEYS) for blk in range(nblk)]
    cap, ksplit, cores = _schedule(slot_lists)

    wqt = _chunkT(Wq, 128)
    wkt = _chunkT(Wk, 128)
    # flat (128, 512): [fh, c] stationary at fh*256 + c*128
    wqs = np.stack([0.5 * WB * wqt, WB * wqt],
                   axis=1).astype(bfd).reshape(128, 512)
    wks = np.stack([0.5 * WB * wkt, WB * wkt],
                   axis=1).astype(bfd).reshape(128, 512)
    al = np.asarray(ALPHA, np.float32) * np.asarray(MFAC, np.float32)
    wvp = np.zeros((128, 4), np.float32)
    wvp[:, :NFREQ] = al[None, :] * Wv[:, None]
    wvn = np.zeros((128, 4), np.float32)
    wvn[:, :NFREQ] = -np.asarray(CCOEF, np.float32)[None, :] \
        * wvp[:, :NFREQ]
    wvp16 = np.ascontiguousarray(wvp).view(np.uint16).view(bfd)  # (128,8)
    wvn16 = np.ascontiguousarray(wvn).view(np.uint16).view(bfd)

    qts = {b: _chunkT(Q[b], 256).astype(bfd).reshape(128, 512)
           for b in range(B)}

    QT_OFF = 1056
    KV_OFF = QT_OFF + 2 * 512
    TOT = KV_OFF + 520 * cap

    in_maps, core_qbs = [], []
    for (ba, sa, bb, sb) in cores:
        slots = list(sa) + [None] * (ksplit - len(sa)) \
            + list(sb) + [None] * ((cap - ksplit) - len(sb))
        core_qbs.append((ba, bb))
        blob = np.zeros((128, TOT), bfd)
        blob[:, 0:512] = wqs
        blob[:, 512:1024] = wks
        blob[:, 1024:1032] = wvp16
        blob[:, 1032:1040] = wvn16
        if ba is not None:
            blob[:, QT_OFF:QT_OFF + 512] = qts[ba]
        if bb is not None:
            blob[:, QT_OFF + 512:QT_OFF + 1024] = qts[bb]
        for si, it in enumerate(slots):
            if it is None:
                continue
            b, j0 = it
            o = KV_OFF + si * 520
            blob[:, o:o + 256] = _chunkT(
                K[b, j0:j0 + SLOT_KEYS],
                SLOT_KEYS).astype(bfd).reshape(128, 256)
            nval = int(np.clip(int(L[b]) - j0, 0, SLOT_KEYS))
            vv = np.zeros((128, VE_W), np.float32)
            vv[:nval, :256] = V[b, j0:j0 + nval]
            vv[:nval, DEN_COL] = 1.0
            blob[:, o + 256:o + 520] = vv.astype(bfd)
        in_maps.append({"blob": blob})
    return cap, ksplit, core_qbs, in_maps


def _gather(core_qbs, results) -> np.ndarray:
    acc = np.zeros((B, NQ, 257), np.float64)
    for c, (ba, bb) in enumerate(core_qbs):
        o = np.asarray(results[c]["out"], np.float64)  # (2,128,2,VE_W)
        for qb, b in ((0, ba), (1, bb)):
            if b is None:
                continue
            # partial[i = ic*128 + p] = o[qb][p, ic]
            acc[b] += o[qb].transpose(1, 0, 2).reshape(NQ, VE_W)[:, :257]
    return (acc[:, :, :256] / acc[:, :, 256:257]).astype(np.float32)


def _install_ntff_hook():
    """Register the axon NTFF profile hook that bass_utils reads via
    antenv.axon_hooks (the shipped antenv stub lacks that module)."""
    import contextlib
    import ctypes
    import sys
    import types

    try:
        from antenv.axon_hooks import get_axon_ntff_profile_hook
        if get_axon_ntff_profile_hook() is not None:
            return
    except ImportError:
        pass

    so_path = "/opt/axon/libaxon_pjrt.so"
    if not os.path.exists(so_path):
        return
    lib = ctypes.CDLL(so_path)
    if not hasattr(lib, "axon_start_nrt_profile"):
        return
    lib.axon_start_nrt_profile.argtypes = [
        ctypes.POINTER(ctypes.c_int64), ctypes.c_size_t]
    lib.axon_start_nrt_profile.restype = ctypes.c_int64
    lib.axon_stop_nrt_profile.argtypes = [ctypes.c_char_p]
    lib.axon_stop_nrt_profile.restype = ctypes.c_int64

    @contextlib.contextmanager
    def _hook(output_dir, device_ids):
        import jax
        jax.devices()
        if device_ids:
            ids = (ctypes.c_int64 * len(device_ids))(*device_ids)
            rc = lib.axon_start_nrt_profile(ids, len(device_ids))
        else:
            rc = lib.axon_start_nrt_profile(None, 0)
        if rc != 0:
            raise RuntimeError(f"axon_start_nrt_profile rc={rc}")
        try:
            yield
        finally:
            n = lib.axon_stop_nrt_profile(str(output_dir).encode())
            print(f"ntff profile: {n} file(s) written to {output_dir}")

    mod = types.ModuleType("antenv.axon_hooks")
    mod.get_axon_ntff_profile_hook = lambda: _hook
    mod.set_axon_ntff_profile_hook = lambda h: None
    sys.modules["antenv.axon_hooks"] = mod
    import antenv
    antenv.axon_hooks = mod


def run(Q_batch, K_batch, V_batch, valid_lens, Wq, Wk, Wv,
        trace: bool = False):
    """Returns (output, exec_time_ns_or_None)."""
    from concourse.bass_utils import run_bass_kernel_spmd

    if trace:
        _install_ntff_hook()

    cap, ksplit, core_qbs, in_maps = _prepare(
        Q_batch, K_batch, V_batch, valid_lens, Wq, Wk, Wv)
    nc = _get_program(cap, ksplit)

    if os.environ.get("ADD_ATTN_SIM"):
        from concourse.bass_interp import CoreSim
        ncores = int(os.environ.get("ADD_ATTN_SIM_CORES", NCORES))
        results = []
        for c in range(ncores):
            sim = CoreSim(nc)
            for name, arr in in_maps[c].items():
                sim.tensor(name)[:] = arr
            sim.simulate()
            results.append({"out": np.array(sim.tensor("out"))})
        return _gather(core_qbs[:ncores], results), None

    res = run_bass_kernel_spmd(nc, in_maps, core_ids=list(range(NCORES)),
                               trace=trace)
    return _gather(core_qbs, res.results), res.exec_time_ns


def kernel(Q_batch, K_batch, V_batch, valid_lens, Wq, Wk, Wv):
    out, _ = run(Q_batch, K_batch, V_batch, valid_lens, Wq, Wk, Wv)
    return out
